# revision 22
# baseline (speedup 1.0000x reference)
"""Trainium2 Bass kernel for causal multi-head attention with RoPE.

Problem: B=4, T=2048, C=1024, 16 heads, head_dim=64, fp32.
Sharding over 8 cores: core c handles batch c//2 and heads [8*(c%2), 8*(c%2)+8).
Each core computes a [T, C] partial of the output projection; the host sums
the two partials per batch and adds b_proj.

v3 schedule:
- phase B: per 128-row x tile: DMA -> 8 PE transposes -> ACT evacuation into
  one big xT buffer -> 8 V matmuls (real matmuls keep HAM warm through the
  transpose stream).
- phase C: Q^T/K^T per 128-channel group (bf16 out) + RoPE on DVE; the whole
  hp=0 attention pass is interleaved between QK chains so exp/AV overlap the
  QKV phase (PSUM banks for S/AV/bc are reserved top-level).
- post C: remaining units ib-major; each ib's projection emitted as soon as
  that ib completes.
- queues: sync = x in + outputs, gpsimd = weights/tables/masks/memsets,
  scalar = psum evacuations + exp only.
"""

import numpy as np
import ml_dtypes
from contextlib import ExitStack

import concourse.bass as bass
import concourse.tile as tile
from concourse import bacc, mybir
from concourse.bass_utils import run_bass_kernel_spmd

F32 = mybir.dt.float32
F32R = mybir.dt.float32r
BF16 = mybir.dt.bfloat16
U32 = mybir.dt.uint32
AF = mybir.ActivationFunctionType

B, T, C = 4, 2048, 1024
N_HEAD = 16
HD = 64  # head dim
HG = 8  # heads per core
DG = HG * HD  # 512 channels per core
NB = 512  # i-block (free dim of S / AV matmuls)
SCALE = 1.0 / np.sqrt(HD)

_NC_CACHE = {}
LAST_RESULTS = None


def _pair_swap_mask():
    m = []
    for i in range(16):
        m += [2 * i + 1, 2 * i]
    return m


def build_nc(t=T):
    key = t
    if key in _NC_CACHE:
        return _NC_CACHE[key]

    n_tt = t // 128  # t tiles of 128
    n_tb = t // NB  # t blocks of 512
    n_ct = C // 128  # contraction tiles over C
    n_dt = DG // 128  # output d tiles (4)
    n_cy = DG // 128  # proj contraction tiles (4)

    nc = bacc.Bacc("TRN2", target_bir_lowering=False, debug=False, num_devices=8)

    x_d = nc.dram_tensor("x", [t, C], BF16, kind="ExternalInput").ap()
    wq_d = nc.dram_tensor("wq", [C, DG], BF16, kind="ExternalInput").ap()
    wk_d = nc.dram_tensor("wk", [C, DG], BF16, kind="ExternalInput").ap()
    wv_d = nc.dram_tensor("wv", [C, DG], BF16, kind="ExternalInput").ap()
    bq_d = nc.dram_tensor("bq", [128, DG // 128], F32, kind="ExternalInput").ap()
    bk_d = nc.dram_tensor("bk", [128, DG // 128], F32, kind="ExternalInput").ap()
    bv_d = nc.dram_tensor("bv", [DG], F32, kind="ExternalInput").ap()
    wp_d = nc.dram_tensor("wp", [DG, C], BF16, kind="ExternalInput").ap()
    cos_d = nc.dram_tensor("cosT", [128, t], F32, kind="ExternalInput").ap()
    sin_d = nc.dram_tensor("sinS", [128, t], F32, kind="ExternalInput").ap()
    out_d = nc.dram_tensor("out", [t, C], F32, kind="ExternalOutput").ap()

    with tile.TileContext(nc) as tc, ExitStack() as ctx:
        # ------- persistent SBUF -------
        persist = ctx.enter_context(tc.tile_pool(name="persist", bufs=1))
        qt_tiles = [persist.tile([128, t], BF16, tag=f"qt{i}", name=f"qt{i}") for i in range(n_dt)]
        kt_tiles = [persist.tile([128, t], BF16, tag=f"kt{i}", name=f"kt{i}") for i in range(n_dt)]
        v_tiles = [
            persist.tile([128, HG * (HD + 1)], BF16, tag=f"v{i}", name=f"v{i}") for i in range(n_tt)
        ]
        yt_tiles = [persist.tile([128, t], BF16, tag=f"yt{i}", name=f"yt{i}") for i in range(n_dt)]
        ones_sb = persist.tile([128, HD], F32R, tag="ones", name="ones")
        nc.vector.memset(ones_sb[:].bitcast(U32), 0x3F800000)
        # pre-fill V tiles with 1.0 so the padding column (softmax denominator
        # ones-row) survives; data columns are overwritten by the V epilogue.
        for vt in v_tiles:
            nc.vector.memset(vt[:].bitcast(U32), 0x3F803F80)
        # exp output ring (bf16 P tiles)
        p_pool = ctx.enter_context(tc.tile_pool(name="p", bufs=6))
        # normalization scratch (lives in both phase C and post-C sections)
        nrm_pool = ctx.enter_context(tc.tile_pool(name="nrm", bufs=4))

        # projection weights (prefetched during phase B)
        wp_pool = ctx.enter_context(tc.tile_pool(name="wp", bufs=1))
        wp_sb = [
            wp_pool.tile([128, C], BF16, tag=f"wp{i}", name=f"wp{i}") for i in range(n_cy)
        ]

        # attention PSUM at top level: S pair ring (4 banks) + AV pair ring
        # (2 banks); phase B/C get the remaining 2 banks.
        ps_att = ctx.enter_context(tc.tile_pool(name="ps_att", bufs=2, space="PSUM"))

        av_cur = {}
        ucur = {"i": 0, "prev": None}

        units = []
        for ib in range(n_tb):
            for jt in range(4 * ib + 4):
                units.append((ib, 0, jt))
        n_hp0 = len(units)
        proj_after = {}
        block_order = [
            (0, 1), (0, 2), (1, 1), (1, 2), (0, 3), (2, 1), (2, 2), (1, 3),
            (3, 1), (3, 2), (2, 3), (3, 3),
        ]
        for ib, hp in block_order:
            for jt in range(4 * ib + 4):
                units.append((ib, hp, jt))
            if hp == 3:
                proj_after[len(units) - 1] = ib

        def emit_s(u):
            ib, hp, jt = units[u]
            sp = ps_att.tile([128, 2 * NB], F32, tag="s", name="s", bufs=2)
            for s in range(2):
                lo = s * HD
                nc.tensor.matmul(
                    sp[:, s * NB : (s + 1) * NB],
                    kt_tiles[hp][lo : lo + HD, jt * 128 : (jt + 1) * 128],
                    qt_tiles[hp][lo : lo + HD, ib * NB : (ib + 1) * NB],
                    start=True,
                    stop=True,
                    tile_position=(lo, 0),
                )
            return sp

        def pair_ap(tl, c0, width=None):
            base = tl[:]
            w = NB - c0 if width is None else width
            return bass.AP(
                tensor=base.tensor,
                offset=base.offset + c0,
                ap=[list(base.ap[0]), [NB, 2], [1, w]],
            )

        def emit_exp_av(u, sp, bc_alloc):
            ib, hp, jt = units[u]
            r = jt - 4 * ib
            c0 = 128 * r if r >= 0 else 0
            n_j = 4 * ib + 4
            if jt == 0:
                av_cur[hp] = [
                    ps_att.tile([HD + 1, NB], F32, tag="av", name="av", bufs=2)
                    for _ in range(2)
                ]
            pt = p_pool.tile([128, 2 * NB], BF16, tag="p", name="p")
            if c0 > 0:
                pb = pt[:].bitcast(U32)
                z = bass.AP(
                    tensor=pb.tensor,
                    offset=pb.offset,
                    ap=[list(pb.ap[0]), [NB // 2, 2], [1, c0 // 2]],
                )
                nc.gpsimd.memset(z, 0)
            nc.scalar.activation(
                pair_ap(pt, c0), pair_ap(sp, c0), AF.Exp, scale=SCALE
            )
            if r >= 0:
                band = min(128, NB - c0)
                nc.gpsimd.affine_select(
                    out=pair_ap(pt, c0, band),
                    in_=pair_ap(pt, c0, band),
                    compare_op=mybir.AluOpType.is_ge,
                    fill=0.0,
                    base=0,
                    pattern=[[0, 2], [1, band]],
                    channel_multiplier=-1,
                )
            for s in range(2):
                h = 2 * hp + s
                nc.tensor.matmul(
                    av_cur[hp][s][:],
                    v_tiles[jt][:, h * (HD + 1) : (h + 1) * (HD + 1)],
                    pt[:, s * NB : (s + 1) * NB],
                    start=(jt == 0),
                    stop=(jt == n_j - 1),
                )
            if jt == n_j - 1:
                for s in range(2):
                    h = 2 * hp + s
                    av = av_cur[hp][s]
                    ytmp = nrm_pool.tile(
                        [HD + 1, NB], F32R, tag="ytmp", name="ytmp"
                    )
                    nc.vector.tensor_copy(ytmp[:], av[:])
                    bc_t = bc_alloc()
                    bc = bc_t[:][0:HD, :]
                    nc.tensor.matmul(
                        bc,
                        ones_sb[HD : HD + 1, :],
                        ytmp[HD : HD + 1, :],
                        start=True,
                        stop=True,
                    )
                    rec = nrm_pool.tile([HD, NB], F32, tag="rec", name="rec")
                    nc.vector.reciprocal_approx_fast(rec[:], bc)
                    dt_i, lo = divmod(h * HD, 128)
                    nc.vector.tensor_mul(
                        yt_tiles[dt_i][lo : lo + HD, ib * NB : (ib + 1) * NB],
                        ytmp[0:HD, :].bitcast(F32),
                        rec[:],
                    )

        def emit_units(n, bc_alloc, max_hp=99):
            # standard software pipeline: S(u+1) issued before exp/AV(u).
            # max_hp gates emission on which RoPE'd qt/kt pairs exist yet in
            # program order (a read emitted before the write reads stale data).
            for _ in range(n):
                u = ucur["i"]
                if u >= len(units) or units[u][1] > max_hp:
                    return
                if ucur["prev"] is None:
                    ucur["prev"] = emit_s(u)
                nxt = None
                if u + 1 < len(units) and units[u + 1][1] <= max_hp:
                    nxt = emit_s(u + 1)
                emit_exp_av(u, ucur["prev"], bc_alloc)
                ucur["prev"] = nxt
                ucur["i"] = u + 1

        # ------- phase B+C pools -------
        with ExitStack() as ph2:
            xt_pool = ph2.enter_context(tc.tile_pool(name="xt", bufs=1))
            xt_all = xt_pool.tile([128, n_ct * t], BF16, tag="xt", name="xt")

            consts = ph2.enter_context(tc.tile_pool(name="consts", bufs=1))
            bq_sb = consts.tile([128, n_dt], F32)
            bk_sb = consts.tile([128, n_dt], F32)
            nc.gpsimd.dma_start(bq_sb[:], bq_d)
            nc.gpsimd.dma_start(bk_sb[:], bk_d)
            bv_sb = consts.tile([128, DG], F32)
            nc.gpsimd.dma_start(
                bv_sb[:],
                bass.AP(tensor=bv_d.tensor, offset=0, ap=[[0, 128], [1, DG]]),
            )

            tab_pool = ph2.enter_context(tc.tile_pool(name="tab", bufs=1))
            cos_sb = tab_pool.tile([128, t], BF16)
            sin_sb = tab_pool.tile([128, t], BF16)

            # QK weight slabs (full width, contiguous rows -> few efficient DMAs)
            wqk_pool = ph2.enter_context(tc.tile_pool(name="wqk", bufs=1))
            wq_sb = [wqk_pool.tile([128, DG], BF16, tag=f"wq{i}", name=f"wq{i}") for i in range(n_ct)]
            wk_sb = [wqk_pool.tile([128, DG], BF16, tag=f"wk{i}", name=f"wk{i}") for i in range(n_ct)]

            # shared F32 psum ring: transposes, V psums, QK psums, phase-C bc
            psW = ph2.enter_context(tc.tile_pool(name="psW", bufs=2, space="PSUM"))

            def bc_alloc_C():
                return psW.tile([128, NB], F32, tag="w", name="bcC")

            # ---- phase B: load x, transpose into xT, V matmuls per tile ----
            with ExitStack() as phB:
                wv_pool = phB.enter_context(tc.tile_pool(name="wv", bufs=1))
                wv_sb = [
                    wv_pool.tile([128, DG], BF16, tag=f"wv{i}", name=f"wv{i}")
                    for i in range(n_ct)
                ]
                for ci in range(n_ct):
                    nc.gpsimd.dma_start(wv_sb[ci][:], wv_d[ci * 128 : (ci + 1) * 128, :])
                # RoPE tables after wv (gpsimd DMA casts f32->bf16)
                nc.gpsimd.dma_start(cos_sb[:], cos_d)
                nc.gpsimd.dma_start(sin_sb[:], sin_d)

                ht = t // 2
                for half in range(2):
                    for ci in range(n_ct):
                        eng = nc.sync
                        eng.dma_start_transpose(
                            xt_all[:, ci * t + half * ht : ci * t + (half + 1) * ht],
                            x_d[half * ht : (half + 1) * ht, ci * 128 : (ci + 1) * 128],
                        )
                    # weight prefetches ride the queues between the halves
                    if half == 0:
                        for ci in range(n_ct):
                            nc.scalar.dma_start(
                                wq_sb[ci][:], wq_d[ci * 128 : (ci + 1) * 128, :]
                            )
                            nc.scalar.dma_start(
                                wk_sb[ci][:], wk_d[ci * 128 : (ci + 1) * 128, :]
                            )
                        for ci in range(n_cy):
                            nc.scalar.dma_start(
                                wp_sb[ci][:], wp_d[ci * 128 : (ci + 1) * 128, :]
                            )
                    for ti in range(half * (n_tt // 2), (half + 1) * (n_tt // 2)):
                        ps = psW.tile([128, DG], F32, tag="w", name="ps2v")
                        for ci in range(n_ct):
                            nc.tensor.matmul(
                                ps[:],
                                xt_all[:, ci * t + ti * 128 : ci * t + (ti + 1) * 128],
                                wv_sb[ci][:],
                                start=(ci == 0),
                                stop=(ci == n_ct - 1),
                            )
                        vt = v_tiles[ti]
                        dst = bass.AP(
                            tensor=vt[:].tensor,
                            offset=vt[:].offset,
                            ap=[list(vt[:].ap[0]), [HD + 1, HG], [1, HD]],
                        )
                        nc.vector.tensor_add(
                            dst,
                            ps[:].rearrange("p (h d) -> p h d", h=HG),
                            bv_sb[:].rearrange("p (h d) -> p h d", h=HG),
                        )

            # ---- phase C: Q^T, K^T + RoPE; hp0 attention pass interleaved ----
            rope_tmp = ph2.enter_context(tc.tile_pool(name="rtmp", bufs=2))
            shuf_mask = _pair_swap_mask()

            def qk_block(w_sb, b_sb, dst, dt_i, units_per_chain, max_hp):
                wts = [
                    w_sb[ci][:, dt_i * 128 : (dt_i + 1) * 128] for ci in range(n_ct)
                ]
                for nb_i in range(n_tb):
                    ps = psW.tile([128, NB], F32, tag="w", name="ps2")
                    for ci in range(n_ct):
                        nc.tensor.matmul(
                            ps[:],
                            wts[ci],
                            xt_all[:, ci * t + nb_i * NB : ci * t + (nb_i + 1) * NB],
                            start=(ci == 0),
                            stop=(ci == n_ct - 1),
                        )
                    nc.vector.tensor_scalar_add(
                        dst[dt_i][:, nb_i * NB : (nb_i + 1) * NB],
                        ps[:],
                        b_sb[:, dt_i : dt_i + 1],
                    )
                    emit_units(units_per_chain, bc_alloc_C, max_hp)

            def rope(q):
                tmp = rope_tmp.tile([128, t], BF16, tag="rtmp", name="rtmp")
                nc.vector.stream_shuffle(
                    tmp[:].bitcast(U32), q[:].bitcast(U32), shuf_mask
                )
                nc.vector.tensor_mul(tmp[:], tmp[:], sin_sb[:])
                nc.vector.tensor_mul(q[:], q[:], cos_sb[:])
                nc.vector.tensor_add(q[:], q[:], tmp[:])

            for dt_i in range(n_dt):
                upc = (0, 2, 3, 1)[dt_i]
                qk_block(wq_sb, bq_sb, qt_tiles, dt_i, upc, dt_i - 1)
                qk_block(wk_sb, bk_sb, kt_tiles, dt_i, upc, dt_i - 1)
                rope(qt_tiles[dt_i])
                rope(kt_tiles[dt_i])
            # drain the rest of the hp0 pass
            emit_units(max(0, n_hp0 - ucur["i"]), bc_alloc_C, 99)

        # ------- post C: remaining units + projection -------
        with ExitStack() as ph3:
            ps_pp = ph3.enter_context(tc.tile_pool(name="ps_pp", bufs=2, space="PSUM"))
            o_pool = ph3.enter_context(tc.tile_pool(name="o", bufs=3))

            def bc_alloc_3():
                return ps_pp.tile([128, NB], F32, tag="pp", name="bc3")

            def emit_proj_chain(ib, k, nb_i):
                ti = ib * (NB // 128) + k
                pp = ps_pp.tile([128, NB], F32, tag="pp", name="pp")
                for ci in range(n_cy):
                    nc.tensor.matmul(
                        pp[:],
                        yt_tiles[ci][:, ti * 128 : (ti + 1) * 128],
                        wp_sb[ci][:, nb_i * NB : (nb_i + 1) * NB],
                        start=(ci == 0),
                        stop=(ci == n_cy - 1),
                    )
                o_sb = o_pool.tile([128, NB], F32, tag="o", name="o")
                nc.vector.tensor_copy(o_sb[:], pp[:])
                nc.sync.dma_start(
                    out_d[
                        ti * 128 : (ti + 1) * 128,
                        nb_i * NB : (nb_i + 1) * NB,
                    ],
                    o_sb[:],
                )

            pending = []
            while ucur["i"] < len(units):
                u = ucur["i"]
                emit_units(1, bc_alloc_3)
                if u in proj_after:
                    ib = proj_after[u]
                    pending.extend(
                        (ib, k, nb_i)
                        for k in range(NB // 128)
                        for nb_i in range(C // NB)
                    )
                if pending:
                    emit_proj_chain(*pending.pop(0))
            for ch in pending:
                emit_proj_chain(*ch)

    nc.compile()
    _NC_CACHE[key] = nc
    return nc


def _rope_tables(t):
    """cos/sin in interleaved layout; sin sign-folded. Matches jax fp32."""
    inv_freq = (
        1.0 / (10000.0 ** (np.arange(0, HD, 2, dtype=np.float32) / np.float32(HD)))
    ).astype(np.float32)
    tt = np.arange(t, dtype=np.float32)
    freqs = (tt[:, None] * inv_freq[None, :]).astype(np.float32)  # [t, 32]
    cos_t = np.cos(freqs).astype(np.float32)  # [t, 32]
    sin_t = np.sin(freqs).astype(np.float32)
    cos64 = np.empty((64, t), dtype=np.float32)
    sinS64 = np.empty((64, t), dtype=np.float32)
    cos64[0::2] = cos_t.T
    cos64[1::2] = cos_t.T
    sinS64[0::2] = -sin_t.T
    sinS64[1::2] = sin_t.T
    cosT = np.concatenate([cos64, cos64], axis=0)  # [128, t]
    sinS = np.concatenate([sinS64, sinS64], axis=0)
    return np.ascontiguousarray(cosT), np.ascontiguousarray(sinS)


def _ilv_perm():
    """Interleave permutation within a head: new[2i]=old[i], new[2i+1]=old[32+i]."""
    p = np.empty(HD, dtype=np.int64)
    p[0::2] = np.arange(32)
    p[1::2] = np.arange(32, 64)
    return p


def kernel(x, w_attn, b_attn, w_proj, b_proj):
    x = np.asarray(x, dtype=np.float32)
    w_attn = np.asarray(w_attn, dtype=np.float32)
    b_attn = np.asarray(b_attn, dtype=np.float32)
    w_proj = np.asarray(w_proj, dtype=np.float32)
    b_proj = np.asarray(b_proj, dtype=np.float32)

    t = x.shape[1]
    nc = build_nc(t)

    ilv = _ilv_perm()
    cosT, sinS = _rope_tables(t)

    in_maps = []
    for c in range(8):
        b = c // 2
        g = c % 2
        heads = np.arange(HG * g, HG * (g + 1))
        qcols = np.concatenate([h * HD + ilv for h in heads])
        wq = np.ascontiguousarray(w_attn[:, qcols]).astype(ml_dtypes.bfloat16)
        wk = np.ascontiguousarray(w_attn[:, C + qcols]).astype(ml_dtypes.bfloat16)
        vcols = np.arange(2 * C + g * DG, 2 * C + (g + 1) * DG)
        wv = np.ascontiguousarray(w_attn[:, vcols]).astype(ml_dtypes.bfloat16)
        bq = np.ascontiguousarray(b_attn[qcols].reshape(-1, 128).T)
        bk = np.ascontiguousarray(b_attn[C + qcols].reshape(-1, 128).T)
        bv = np.ascontiguousarray(b_attn[vcols])
        wp = np.ascontiguousarray(w_proj[g * DG : (g + 1) * DG, :]).astype(ml_dtypes.bfloat16)
        in_maps.append(
            {
                "x": np.ascontiguousarray(x[b]).astype(ml_dtypes.bfloat16),
                "wq": wq,
                "wk": wk,
                "wv": wv,
                "bq": bq,
                "bk": bk,
                "bv": bv,
                "wp": wp,
                "cosT": cosT,
                "sinS": sinS,
            }
        )

    res = run_bass_kernel_spmd(nc, in_maps, core_ids=list(range(8)))
    global LAST_RESULTS
    LAST_RESULTS = res

    out = np.empty((B, t, C), dtype=np.float32)
    for b in range(B):
        acc = (
            res.results[2 * b]["out"].astype(np.float64)
            + res.results[2 * b + 1]["out"].astype(np.float64)
            + b_proj.astype(np.float64)[None, :]
        )
        out[b] = acc.astype(np.float32)
    return out


# revision 23
# speedup vs baseline: 1.1031x; 1.1031x over previous
"""Trainium2 Bass kernel for causal multi-head attention with RoPE.

Problem: B=4, T=2048, C=1024, 16 heads, head_dim=64, fp32.
Sharding over 8 cores: core c handles batch c//2 and heads [8*(c%2), 8*(c%2)+8).
Each core computes a [T, C] partial of the output projection; the host sums
the two partials per batch and adds b_proj.

v3 schedule:
- phase B: per 128-row x tile: DMA -> 8 PE transposes -> ACT evacuation into
  one big xT buffer -> 8 V matmuls (real matmuls keep HAM warm through the
  transpose stream).
- phase C: Q^T/K^T per 128-channel group (bf16 out) + RoPE on DVE; the whole
  hp=0 attention pass is interleaved between QK chains so exp/AV overlap the
  QKV phase (PSUM banks for S/AV/bc are reserved top-level).
- post C: remaining units ib-major; each ib's projection emitted as soon as
  that ib completes.
- queues: sync = x in + outputs, gpsimd = weights/tables/masks/memsets,
  scalar = psum evacuations + exp only.
"""

import numpy as np
import ml_dtypes
from contextlib import ExitStack

import concourse.bass as bass
import concourse.tile as tile
from concourse import bacc, mybir
from concourse.bass_utils import run_bass_kernel_spmd

F32 = mybir.dt.float32
F32R = mybir.dt.float32r
BF16 = mybir.dt.bfloat16
U32 = mybir.dt.uint32
AF = mybir.ActivationFunctionType

B, T, C = 4, 2048, 1024
N_HEAD = 16
HD = 64  # head dim
HG = 8  # heads per core
DG = HG * HD  # 512 channels per core
NB = 512  # i-block (free dim of S / AV matmuls)
SCALE = 1.0 / np.sqrt(HD)

_NC_CACHE = {}
LAST_RESULTS = None


def _pair_swap_mask():
    m = []
    for i in range(16):
        m += [2 * i + 1, 2 * i]
    return m


def build_nc(t=T):
    key = t
    if key in _NC_CACHE:
        return _NC_CACHE[key]

    n_tt = t // 128  # t tiles of 128
    n_tb = t // NB  # t blocks of 512
    n_ct = C // 128  # contraction tiles over C
    n_dt = DG // 128  # output d tiles (4)
    n_cy = DG // 128  # proj contraction tiles (4)

    nc = bacc.Bacc("TRN2", target_bir_lowering=False, debug=False, num_devices=8)

    x_d = nc.dram_tensor("x", [t, C], F32, kind="ExternalInput").ap()
    wq_d = nc.dram_tensor("wq", [C, DG], BF16, kind="ExternalInput").ap()
    wk_d = nc.dram_tensor("wk", [C, DG], BF16, kind="ExternalInput").ap()
    wv_d = nc.dram_tensor("wv", [C, DG], BF16, kind="ExternalInput").ap()
    bq_d = nc.dram_tensor("bq", [128, DG // 128], F32, kind="ExternalInput").ap()
    bk_d = nc.dram_tensor("bk", [128, DG // 128], F32, kind="ExternalInput").ap()
    bv_d = nc.dram_tensor("bv", [DG], F32, kind="ExternalInput").ap()
    wp_d = nc.dram_tensor("wp", [DG, C], BF16, kind="ExternalInput").ap()
    cos_d = nc.dram_tensor("cosT", [128, t], F32, kind="ExternalInput").ap()
    sin_d = nc.dram_tensor("sinS", [128, t], F32, kind="ExternalInput").ap()
    out_d = nc.dram_tensor("out", [t, C], F32, kind="ExternalOutput").ap()

    with tile.TileContext(nc) as tc, ExitStack() as ctx:
        # ------- persistent SBUF -------
        persist = ctx.enter_context(tc.tile_pool(name="persist", bufs=1))
        qt_tiles = [persist.tile([128, t], BF16, tag=f"qt{i}", name=f"qt{i}") for i in range(n_dt)]
        kt_tiles = [persist.tile([128, t], BF16, tag=f"kt{i}", name=f"kt{i}") for i in range(n_dt)]
        v_tiles = [
            persist.tile([128, HG * (HD + 1)], BF16, tag=f"v{i}", name=f"v{i}") for i in range(n_tt)
        ]
        yt_tiles = [persist.tile([128, t], BF16, tag=f"yt{i}", name=f"yt{i}") for i in range(n_dt)]
        ones_sb = persist.tile([128, HD], F32R, tag="ones", name="ones")
        nc.vector.memset(ones_sb[:].bitcast(U32), 0x3F800000)
        # pre-fill V tiles with 1.0 so the padding column (softmax denominator
        # ones-row) survives; data columns are overwritten by the V epilogue.
        for vt in v_tiles:
            nc.vector.memset(vt[:].bitcast(U32), 0x3F803F80)
        # exp output ring (bf16 P tiles)
        p_pool = ctx.enter_context(tc.tile_pool(name="p", bufs=6))
        # normalization scratch (lives in both phase C and post-C sections)
        nrm_pool = ctx.enter_context(tc.tile_pool(name="nrm", bufs=4))

        # projection weights (prefetched during phase B)
        wp_pool = ctx.enter_context(tc.tile_pool(name="wp", bufs=1))
        wp_sb = [
            wp_pool.tile([128, C], BF16, tag=f"wp{i}", name=f"wp{i}") for i in range(n_cy)
        ]

        # attention PSUM at top level: S pair ring (4 banks) + AV pair ring
        # (2 banks); phase B/C get the remaining 2 banks.
        ps_att = ctx.enter_context(tc.tile_pool(name="ps_att", bufs=2, space="PSUM"))

        av_cur = {}
        ucur = {"i": 0, "prev": None}

        units = []
        for ib in range(n_tb):
            for jt in range(4 * ib + 4):
                units.append((ib, 0, jt))
        n_hp0 = len(units)
        proj_after = {}
        block_order = [
            (0, 1), (0, 2), (1, 1), (1, 2), (0, 3), (2, 1), (2, 2), (1, 3),
            (3, 1), (3, 2), (2, 3), (3, 3),
        ]
        for ib, hp in block_order:
            for jt in range(4 * ib + 4):
                units.append((ib, hp, jt))
            if hp == 3:
                proj_after[len(units) - 1] = ib

        def emit_s(u):
            ib, hp, jt = units[u]
            sp = ps_att.tile([128, 2 * NB], F32, tag="s", name="s", bufs=2)
            for s in range(2):
                lo = s * HD
                nc.tensor.matmul(
                    sp[:, s * NB : (s + 1) * NB],
                    kt_tiles[hp][lo : lo + HD, jt * 128 : (jt + 1) * 128],
                    qt_tiles[hp][lo : lo + HD, ib * NB : (ib + 1) * NB],
                    start=True,
                    stop=True,
                    tile_position=(lo, 0),
                )
            return sp

        def pair_ap(tl, c0, width=None):
            base = tl[:]
            w = NB - c0 if width is None else width
            return bass.AP(
                tensor=base.tensor,
                offset=base.offset + c0,
                ap=[list(base.ap[0]), [NB, 2], [1, w]],
            )

        def emit_exp_av(u, sp, bc_alloc):
            ib, hp, jt = units[u]
            r = jt - 4 * ib
            c0 = 128 * r if r >= 0 else 0
            n_j = 4 * ib + 4
            if jt == 0:
                av_cur[hp] = [
                    ps_att.tile([HD + 1, NB], F32, tag="av", name="av", bufs=2)
                    for _ in range(2)
                ]
            pt = p_pool.tile([128, 2 * NB], BF16, tag="p", name="p")
            if c0 > 0:
                pb = pt[:].bitcast(U32)
                z = bass.AP(
                    tensor=pb.tensor,
                    offset=pb.offset,
                    ap=[list(pb.ap[0]), [NB // 2, 2], [1, c0 // 2]],
                )
                nc.gpsimd.memset(z, 0)
            nc.scalar.activation(
                pair_ap(pt, c0), pair_ap(sp, c0), AF.Exp, scale=SCALE
            )
            if r >= 0:
                band = min(128, NB - c0)
                nc.gpsimd.affine_select(
                    out=pair_ap(pt, c0, band),
                    in_=pair_ap(pt, c0, band),
                    compare_op=mybir.AluOpType.is_ge,
                    fill=0.0,
                    base=0,
                    pattern=[[0, 2], [1, band]],
                    channel_multiplier=-1,
                )
            for s in range(2):
                h = 2 * hp + s
                nc.tensor.matmul(
                    av_cur[hp][s][:],
                    v_tiles[jt][:, h * (HD + 1) : (h + 1) * (HD + 1)],
                    pt[:, s * NB : (s + 1) * NB],
                    start=(jt == 0),
                    stop=(jt == n_j - 1),
                )
            if jt == n_j - 1:
                for s in range(2):
                    h = 2 * hp + s
                    av = av_cur[hp][s]
                    ytmp = nrm_pool.tile(
                        [HD + 1, NB], F32R, tag="ytmp", name="ytmp"
                    )
                    nc.vector.tensor_copy(ytmp[:], av[:])
                    bc_t = bc_alloc()
                    bc = bc_t[:][0:HD, :]
                    nc.tensor.matmul(
                        bc,
                        ones_sb[HD : HD + 1, :],
                        ytmp[HD : HD + 1, :],
                        start=True,
                        stop=True,
                    )
                    rec = nrm_pool.tile([HD, NB], F32, tag="rec", name="rec")
                    nc.vector.reciprocal_approx_fast(rec[:], bc)
                    dt_i, lo = divmod(h * HD, 128)
                    nc.vector.tensor_mul(
                        yt_tiles[dt_i][lo : lo + HD, ib * NB : (ib + 1) * NB],
                        ytmp[0:HD, :].bitcast(F32),
                        rec[:],
                    )

        def emit_units(n, bc_alloc, max_hp=99):
            # standard software pipeline: S(u+1) issued before exp/AV(u).
            # max_hp gates emission on which RoPE'd qt/kt pairs exist yet in
            # program order (a read emitted before the write reads stale data).
            for _ in range(n):
                u = ucur["i"]
                if u >= len(units) or units[u][1] > max_hp:
                    return
                if ucur["prev"] is None:
                    ucur["prev"] = emit_s(u)
                nxt = None
                if u + 1 < len(units) and units[u + 1][1] <= max_hp:
                    nxt = emit_s(u + 1)
                emit_exp_av(u, ucur["prev"], bc_alloc)
                ucur["prev"] = nxt
                ucur["i"] = u + 1

        # ------- phase B+C pools -------
        with ExitStack() as ph2:
            xt_pool = ph2.enter_context(tc.tile_pool(name="xt", bufs=1))
            xt_all = xt_pool.tile([128, n_ct * t], BF16, tag="xt", name="xt")

            consts = ph2.enter_context(tc.tile_pool(name="consts", bufs=1))
            ident = consts.tile([128, 128], F32)
            nc.vector.memset(ident[:].bitcast(U32), 0)
            nc.gpsimd.affine_select(
                out=ident[:],
                in_=ident[:],
                compare_op=mybir.AluOpType.not_equal,
                fill=1.0,
                base=0,
                pattern=[[-1, 128]],
                channel_multiplier=1,
            )
            bq_sb = consts.tile([128, n_dt], F32)
            bk_sb = consts.tile([128, n_dt], F32)
            nc.gpsimd.dma_start(bq_sb[:], bq_d)
            nc.gpsimd.dma_start(bk_sb[:], bk_d)
            bv_sb = consts.tile([128, DG], F32)
            nc.gpsimd.dma_start(
                bv_sb[:],
                bass.AP(tensor=bv_d.tensor, offset=0, ap=[[0, 128], [1, DG]]),
            )

            tab_pool = ph2.enter_context(tc.tile_pool(name="tab", bufs=1))
            cos_sb = tab_pool.tile([128, t], BF16)
            sin_sb = tab_pool.tile([128, t], BF16)

            # QK weight slabs (full width, contiguous rows -> few efficient DMAs)
            wqk_pool = ph2.enter_context(tc.tile_pool(name="wqk", bufs=1))
            wq_sb = [wqk_pool.tile([128, DG], BF16, tag=f"wq{i}", name=f"wq{i}") for i in range(n_ct)]
            wk_sb = [wqk_pool.tile([128, DG], BF16, tag=f"wk{i}", name=f"wk{i}") for i in range(n_ct)]

            # shared F32 psum ring: transposes, V psums, QK psums, phase-C bc
            psW = ph2.enter_context(tc.tile_pool(name="psW", bufs=2, space="PSUM"))
            xa_pool = ph2.enter_context(tc.tile_pool(name="xa", bufs=3))

            def bc_alloc_C():
                return psW.tile([128, NB], F32, tag="w", name="bcC")

            # ---- phase B: load x, transpose into xT, V matmuls per tile ----
            with ExitStack() as phB:
                wv_pool = phB.enter_context(tc.tile_pool(name="wv", bufs=1))
                wv_sb = [
                    wv_pool.tile([128, DG], BF16, tag=f"wv{i}", name=f"wv{i}")
                    for i in range(n_ct)
                ]
                for ci in range(n_ct):
                    nc.gpsimd.dma_start(wv_sb[ci][:], wv_d[ci * 128 : (ci + 1) * 128, :])
                # RoPE tables after wv (gpsimd DMA casts f32->bf16)
                nc.gpsimd.dma_start(cos_sb[:], cos_d)
                nc.gpsimd.dma_start(sin_sb[:], sin_d)

                for ti in range(n_tt):
                    xa = xa_pool.tile([128, C], F32, tag="xa", name="xa")
                    nc.sync.dma_start(xa[:], x_d[ti * 128 : (ti + 1) * 128, :])
                    # spread weight prefetches across phase B on the sync queue
                    if ti < 8:
                        nc.sync.dma_start(
                            wq_sb[ti][:], wq_d[ti * 128 : (ti + 1) * 128, :]
                        )
                        nc.sync.dma_start(
                            wk_sb[ti][:], wk_d[ti * 128 : (ti + 1) * 128, :]
                        )
                    elif ti < 12:
                        ci = ti - 8
                        nc.sync.dma_start(
                            wp_sb[ci][:], wp_d[ci * 128 : (ci + 1) * 128, :]
                        )
                    for half in range(2):
                        tp = psW.tile([128, 512], F32, tag="w", name="tp")
                        for k in range(4):
                            ci = half * 4 + k
                            nc.tensor.transpose(
                                tp[:, k * 128 : (k + 1) * 128],
                                xa[:, ci * 128 : (ci + 1) * 128],
                                ident[:],
                            )
                        base = xt_all[:]
                        dst = bass.AP(
                            tensor=base.tensor,
                            offset=base.offset + (half * 4) * t + ti * 128,
                            ap=[list(base.ap[0]), [t, 4], [1, 128]],
                        )
                        nc.scalar.copy(
                            dst, tp[:].rearrange("p (g c) -> p g c", g=4)
                        )
                    ps = psW.tile([128, DG], F32, tag="w", name="ps2v")
                    for ci in range(n_ct):
                        nc.tensor.matmul(
                            ps[:],
                            xt_all[:, ci * t + ti * 128 : ci * t + (ti + 1) * 128],
                            wv_sb[ci][:],
                            start=(ci == 0),
                            stop=(ci == n_ct - 1),
                        )
                    vt = v_tiles[ti]
                    dst = bass.AP(
                        tensor=vt[:].tensor,
                        offset=vt[:].offset,
                        ap=[list(vt[:].ap[0]), [HD + 1, HG], [1, HD]],
                    )
                    nc.vector.tensor_add(
                        dst,
                        ps[:].rearrange("p (h d) -> p h d", h=HG),
                        bv_sb[:].rearrange("p (h d) -> p h d", h=HG),
                    )

            # ---- phase C: Q^T, K^T + RoPE; hp0 attention pass interleaved ----
            rope_tmp = ph2.enter_context(tc.tile_pool(name="rtmp", bufs=2))
            shuf_mask = _pair_swap_mask()

            def qk_block(w_sb, b_sb, dst, dt_i, units_per_chain, max_hp):
                wts = [
                    w_sb[ci][:, dt_i * 128 : (dt_i + 1) * 128] for ci in range(n_ct)
                ]
                for nb_i in range(n_tb):
                    ps = psW.tile([128, NB], F32, tag="w", name="ps2")
                    for ci in range(n_ct):
                        nc.tensor.matmul(
                            ps[:],
                            wts[ci],
                            xt_all[:, ci * t + nb_i * NB : ci * t + (nb_i + 1) * NB],
                            start=(ci == 0),
                            stop=(ci == n_ct - 1),
                        )
                    nc.vector.tensor_scalar_add(
                        dst[dt_i][:, nb_i * NB : (nb_i + 1) * NB],
                        ps[:],
                        b_sb[:, dt_i : dt_i + 1],
                    )
                    emit_units(units_per_chain, bc_alloc_C, max_hp)

            def rope(q):
                tmp = rope_tmp.tile([128, t], BF16, tag="rtmp", name="rtmp")
                nc.vector.stream_shuffle(
                    tmp[:].bitcast(U32), q[:].bitcast(U32), shuf_mask
                )
                nc.vector.tensor_mul(tmp[:], tmp[:], sin_sb[:])
                nc.vector.tensor_mul(q[:], q[:], cos_sb[:])
                nc.vector.tensor_add(q[:], q[:], tmp[:])

            for dt_i in range(n_dt):
                upc = (0, 2, 3, 1)[dt_i]
                qk_block(wq_sb, bq_sb, qt_tiles, dt_i, upc, dt_i - 1)
                qk_block(wk_sb, bk_sb, kt_tiles, dt_i, upc, dt_i - 1)
                rope(qt_tiles[dt_i])
                rope(kt_tiles[dt_i])
            # drain the rest of the hp0 pass
            emit_units(max(0, n_hp0 - ucur["i"]), bc_alloc_C, 99)

        # ------- post C: remaining units + projection -------
        with ExitStack() as ph3:
            ps_pp = ph3.enter_context(tc.tile_pool(name="ps_pp", bufs=2, space="PSUM"))
            o_pool = ph3.enter_context(tc.tile_pool(name="o", bufs=3))

            def bc_alloc_3():
                return ps_pp.tile([128, NB], F32, tag="pp", name="bc3")

            def emit_proj_chain(ib, k, nb_i):
                ti = ib * (NB // 128) + k
                pp = ps_pp.tile([128, NB], F32, tag="pp", name="pp")
                for ci in range(n_cy):
                    nc.tensor.matmul(
                        pp[:],
                        yt_tiles[ci][:, ti * 128 : (ti + 1) * 128],
                        wp_sb[ci][:, nb_i * NB : (nb_i + 1) * NB],
                        start=(ci == 0),
                        stop=(ci == n_cy - 1),
                    )
                o_sb = o_pool.tile([128, NB], F32, tag="o", name="o")
                nc.vector.tensor_copy(o_sb[:], pp[:])
                nc.sync.dma_start(
                    out_d[
                        ti * 128 : (ti + 1) * 128,
                        nb_i * NB : (nb_i + 1) * NB,
                    ],
                    o_sb[:],
                )

            pending = []
            while ucur["i"] < len(units):
                u = ucur["i"]
                emit_units(1, bc_alloc_3)
                if u in proj_after:
                    ib = proj_after[u]
                    pending.extend(
                        (ib, k, nb_i)
                        for k in range(NB // 128)
                        for nb_i in range(C // NB)
                    )
                if pending:
                    emit_proj_chain(*pending.pop(0))
            for ch in pending:
                emit_proj_chain(*ch)

    nc.compile()
    _NC_CACHE[key] = nc
    return nc


def _rope_tables(t):
    """cos/sin in interleaved layout; sin sign-folded. Matches jax fp32."""
    inv_freq = (
        1.0 / (10000.0 ** (np.arange(0, HD, 2, dtype=np.float32) / np.float32(HD)))
    ).astype(np.float32)
    tt = np.arange(t, dtype=np.float32)
    freqs = (tt[:, None] * inv_freq[None, :]).astype(np.float32)  # [t, 32]
    cos_t = np.cos(freqs).astype(np.float32)  # [t, 32]
    sin_t = np.sin(freqs).astype(np.float32)
    cos64 = np.empty((64, t), dtype=np.float32)
    sinS64 = np.empty((64, t), dtype=np.float32)
    cos64[0::2] = cos_t.T
    cos64[1::2] = cos_t.T
    sinS64[0::2] = -sin_t.T
    sinS64[1::2] = sin_t.T
    cosT = np.concatenate([cos64, cos64], axis=0)  # [128, t]
    sinS = np.concatenate([sinS64, sinS64], axis=0)
    return np.ascontiguousarray(cosT), np.ascontiguousarray(sinS)


def _ilv_perm():
    """Interleave permutation within a head: new[2i]=old[i], new[2i+1]=old[32+i]."""
    p = np.empty(HD, dtype=np.int64)
    p[0::2] = np.arange(32)
    p[1::2] = np.arange(32, 64)
    return p


def kernel(x, w_attn, b_attn, w_proj, b_proj):
    x = np.asarray(x, dtype=np.float32)
    w_attn = np.asarray(w_attn, dtype=np.float32)
    b_attn = np.asarray(b_attn, dtype=np.float32)
    w_proj = np.asarray(w_proj, dtype=np.float32)
    b_proj = np.asarray(b_proj, dtype=np.float32)

    t = x.shape[1]
    nc = build_nc(t)

    ilv = _ilv_perm()
    cosT, sinS = _rope_tables(t)

    in_maps = []
    for c in range(8):
        b = c // 2
        g = c % 2
        heads = np.arange(HG * g, HG * (g + 1))
        qcols = np.concatenate([h * HD + ilv for h in heads])
        wq = np.ascontiguousarray(w_attn[:, qcols]).astype(ml_dtypes.bfloat16)
        wk = np.ascontiguousarray(w_attn[:, C + qcols]).astype(ml_dtypes.bfloat16)
        vcols = np.arange(2 * C + g * DG, 2 * C + (g + 1) * DG)
        wv = np.ascontiguousarray(w_attn[:, vcols]).astype(ml_dtypes.bfloat16)
        bq = np.ascontiguousarray(b_attn[qcols].reshape(-1, 128).T)
        bk = np.ascontiguousarray(b_attn[C + qcols].reshape(-1, 128).T)
        bv = np.ascontiguousarray(b_attn[vcols])
        wp = np.ascontiguousarray(w_proj[g * DG : (g + 1) * DG, :]).astype(ml_dtypes.bfloat16)
        in_maps.append(
            {
                "x": np.ascontiguousarray(x[b]),
                "wq": wq,
                "wk": wk,
                "wv": wv,
                "bq": bq,
                "bk": bk,
                "bv": bv,
                "wp": wp,
                "cosT": cosT,
                "sinS": sinS,
            }
        )

    res = run_bass_kernel_spmd(nc, in_maps, core_ids=list(range(8)))
    global LAST_RESULTS
    LAST_RESULTS = res

    out = np.empty((B, t, C), dtype=np.float32)
    for b in range(B):
        acc = (
            res.results[2 * b]["out"].astype(np.float64)
            + res.results[2 * b + 1]["out"].astype(np.float64)
            + b_proj.astype(np.float64)[None, :]
        )
        out[b] = acc.astype(np.float32)
    return out


# revision 25
# speedup vs baseline: 1.1176x; 1.0131x over previous
"""Trainium2 Bass kernel for causal multi-head attention with RoPE.

Problem: B=4, T=2048, C=1024, 16 heads, head_dim=64, fp32.
Sharding over 8 cores: core c handles batch c//2 and heads [8*(c%2), 8*(c%2)+8).
Each core computes a [T, C] partial of the output projection; the host sums
the two partials per batch and adds b_proj.

Schedule:
- phase B: per 128-row x tile: DMA -> 8 PE transposes -> ACT evacuation (with
  f32->bf16 cast) into one big xT buffer -> 8 V matmuls; the V matmuls keep
  the PE's HAM clock-gate warm through the (HAM-invisible) transpose stream.
  QK/proj weight slabs prefetch on the sync queue between x tiles.
- phase C: Q^T/K^T per 128-channel group (bf16) + RoPE on DVE (u32-bitcast
  pair shuffle + 2x-mode bf16 muls); the whole hp=0 attention pass plus the
  first ib-major blocks are interleaved between QK chains (gated on which
  RoPE pairs exist in program order) so exp/AV overlap the QKV phase.
- post C: remaining (ib, hp) blocks in a staggered order that keeps ready
  work ahead of blocks waiting on the last RoPE; each ib's projection chains
  interleave one-per-unit so exp never starves.
- softmax: exp (no max-subtraction; inputs are well-scaled), denominator via
  a ones-row appended to V (AV matmul computes sums for free), reciprocal on
  DVE, normalization fused into the y^T write.
- PSUM: S-pair ring (4 banks) + AV pair (2) top-level; one shared F32 ring
  (2) serves transposes/V/QK/phase-C bc; proj/bc ring (2) post-C.
- queues: sync = x + weight slabs + outputs, gpsimd = tables/biases/wv/masks,
  scalar = psum evacuations + exp only.
"""

import numpy as np
import ml_dtypes
from contextlib import ExitStack

import concourse.bass as bass
import concourse.tile as tile
from concourse import bacc, mybir
from concourse.bass_utils import run_bass_kernel_spmd

F32 = mybir.dt.float32
F32R = mybir.dt.float32r
BF16 = mybir.dt.bfloat16
U32 = mybir.dt.uint32
AF = mybir.ActivationFunctionType

B, T, C = 4, 2048, 1024
N_HEAD = 16
HD = 64  # head dim
HG = 8  # heads per core
DG = HG * HD  # 512 channels per core
NB = 512  # i-block (free dim of S / AV matmuls)
SCALE = 1.0 / np.sqrt(HD)

_NC_CACHE = {}
LAST_RESULTS = None


def _pair_swap_mask():
    m = []
    for i in range(16):
        m += [2 * i + 1, 2 * i]
    return m


def build_nc(t=T):
    key = t
    if key in _NC_CACHE:
        return _NC_CACHE[key]

    n_tt = t // 128  # t tiles of 128
    n_tb = t // NB  # t blocks of 512
    n_ct = C // 128  # contraction tiles over C
    n_dt = DG // 128  # output d tiles (4)
    n_cy = DG // 128  # proj contraction tiles (4)

    nc = bacc.Bacc("TRN2", target_bir_lowering=False, debug=False, num_devices=8)

    x_d = nc.dram_tensor("x", [t, C], F32, kind="ExternalInput").ap()
    wq_d = nc.dram_tensor("wq", [C, DG], BF16, kind="ExternalInput").ap()
    wk_d = nc.dram_tensor("wk", [C, DG], BF16, kind="ExternalInput").ap()
    wv_d = nc.dram_tensor("wv", [C, DG], BF16, kind="ExternalInput").ap()
    bq_d = nc.dram_tensor("bq", [128, DG // 128], F32, kind="ExternalInput").ap()
    bk_d = nc.dram_tensor("bk", [128, DG // 128], F32, kind="ExternalInput").ap()
    bv_d = nc.dram_tensor("bv", [DG], F32, kind="ExternalInput").ap()
    wp_d = nc.dram_tensor("wp", [DG, C], BF16, kind="ExternalInput").ap()
    cos_d = nc.dram_tensor("cosT", [128, t], F32, kind="ExternalInput").ap()
    sin_d = nc.dram_tensor("sinS", [128, t], F32, kind="ExternalInput").ap()
    out_d = nc.dram_tensor("out", [t, C], F32, kind="ExternalOutput").ap()

    with tile.TileContext(nc) as tc, ExitStack() as ctx:
        # ------- persistent SBUF -------
        persist = ctx.enter_context(tc.tile_pool(name="persist", bufs=1))
        qt_tiles = [persist.tile([128, t], BF16, tag=f"qt{i}", name=f"qt{i}") for i in range(n_dt)]
        kt_tiles = [persist.tile([128, t], BF16, tag=f"kt{i}", name=f"kt{i}") for i in range(n_dt)]
        v_tiles = [
            persist.tile([128, HG * (HD + 1)], BF16, tag=f"v{i}", name=f"v{i}") for i in range(n_tt)
        ]
        yt_tiles = [persist.tile([128, t], BF16, tag=f"yt{i}", name=f"yt{i}") for i in range(n_dt)]
        ones_sb = persist.tile([128, HD], F32R, tag="ones", name="ones")
        # exp output ring (bf16 P tiles)
        p_pool = ctx.enter_context(tc.tile_pool(name="p", bufs=6))
        # normalization scratch (lives in both phase C and post-C sections)
        nrm_pool = ctx.enter_context(tc.tile_pool(name="nrm", bufs=4))

        # projection weights (prefetched during phase B)
        wp_pool = ctx.enter_context(tc.tile_pool(name="wp", bufs=1))
        wp_sb = [
            wp_pool.tile([128, C], BF16, tag=f"wp{i}", name=f"wp{i}") for i in range(n_cy)
        ]

        # attention PSUM at top level: S pair ring (4 banks) + AV pair ring
        # (2 banks); phase B/C get the remaining 2 banks.
        ps_att = ctx.enter_context(tc.tile_pool(name="ps_att", bufs=2, space="PSUM"))

        av_cur = {}
        ucur = {"i": 0, "prev": None}

        units = []
        for ib in range(n_tb):
            for jt in range(4 * ib + 4):
                units.append((ib, 0, jt))
        n_hp0 = len(units)
        proj_after = {}
        block_order = [
            (0, 1), (0, 2), (1, 1), (1, 2), (0, 3), (2, 1), (2, 2), (1, 3),
            (3, 1), (3, 2), (2, 3), (3, 3),
        ]
        for ib, hp in block_order:
            for jt in range(4 * ib + 4):
                units.append((ib, hp, jt))
            if hp == 3:
                proj_after[len(units) - 1] = ib

        def emit_s(u):
            ib, hp, jt = units[u]
            sp = ps_att.tile([128, 2 * NB], F32, tag="s", name="s", bufs=2)
            for s in range(2):
                lo = s * HD
                nc.tensor.matmul(
                    sp[:, s * NB : (s + 1) * NB],
                    kt_tiles[hp][lo : lo + HD, jt * 128 : (jt + 1) * 128],
                    qt_tiles[hp][lo : lo + HD, ib * NB : (ib + 1) * NB],
                    start=True,
                    stop=True,
                    tile_position=(lo, 0),
                )
            return sp

        def pair_ap(tl, c0, width=None):
            base = tl[:]
            w = NB - c0 if width is None else width
            return bass.AP(
                tensor=base.tensor,
                offset=base.offset + c0,
                ap=[list(base.ap[0]), [NB, 2], [1, w]],
            )

        def emit_exp_av(u, sp, bc_alloc, filler=None):
            ib, hp, jt = units[u]
            r = jt - 4 * ib
            c0 = 128 * r if r >= 0 else 0
            n_j = 4 * ib + 4
            if jt == 0:
                av_cur[hp] = [
                    ps_att.tile([HD + 1, NB], F32, tag="av", name="av", bufs=2)
                    for _ in range(2)
                ]
            pt = p_pool.tile([128, 2 * NB], BF16, tag="p", name="p")
            if c0 > 0:
                pb = pt[:].bitcast(U32)
                z = bass.AP(
                    tensor=pb.tensor,
                    offset=pb.offset,
                    ap=[list(pb.ap[0]), [NB // 2, 2], [1, c0 // 2]],
                )
                nc.gpsimd.memset(z, 0)
            nc.scalar.activation(
                pair_ap(pt, c0), pair_ap(sp, c0), AF.Exp, scale=SCALE
            )
            if r >= 0:
                band = min(128, NB - c0)
                nc.gpsimd.affine_select(
                    out=pair_ap(pt, c0, band),
                    in_=pair_ap(pt, c0, band),
                    compare_op=mybir.AluOpType.is_ge,
                    fill=0.0,
                    base=0,
                    pattern=[[0, 2], [1, band]],
                    channel_multiplier=-1,
                )
            if filler is not None:
                filler()
            for s in range(2):
                h = 2 * hp + s
                nc.tensor.matmul(
                    av_cur[hp][s][:],
                    v_tiles[jt][:, h * (HD + 1) : (h + 1) * (HD + 1)],
                    pt[:, s * NB : (s + 1) * NB],
                    start=(jt == 0),
                    stop=(jt == n_j - 1),
                )
            if jt == n_j - 1:
                for s in range(2):
                    h = 2 * hp + s
                    av = av_cur[hp][s]
                    ytmp = nrm_pool.tile(
                        [HD + 1, NB], F32R, tag="ytmp", name="ytmp"
                    )
                    nc.vector.tensor_copy(ytmp[:], av[:])
                    bc_t = bc_alloc()
                    bc = bc_t[:][0:HD, :]
                    nc.tensor.matmul(
                        bc,
                        ones_sb[HD : HD + 1, :],
                        ytmp[HD : HD + 1, :],
                        start=True,
                        stop=True,
                    )
                    rec = nrm_pool.tile([HD, NB], F32, tag="rec", name="rec")
                    nc.vector.reciprocal_approx_fast(rec[:], bc)
                    dt_i, lo = divmod(h * HD, 128)
                    nc.vector.tensor_mul(
                        yt_tiles[dt_i][lo : lo + HD, ib * NB : (ib + 1) * NB],
                        ytmp[0:HD, :].bitcast(F32),
                        rec[:],
                    )

        def emit_units(n, bc_alloc, max_hp=99, filler=None):
            # standard software pipeline: S(u+1) issued before exp/AV(u).
            # max_hp gates emission on which RoPE'd qt/kt pairs exist yet in
            # program order (a read emitted before the write reads stale data).
            for _ in range(n):
                u = ucur["i"]
                if u >= len(units) or units[u][1] > max_hp:
                    return
                if ucur["prev"] is None:
                    ucur["prev"] = emit_s(u)
                nxt = None
                if u + 1 < len(units) and units[u + 1][1] <= max_hp:
                    nxt = emit_s(u + 1)
                emit_exp_av(u, ucur["prev"], bc_alloc, filler)
                ucur["prev"] = nxt
                ucur["i"] = u + 1

        # ------- phase B+C pools -------
        with ExitStack() as ph2:
            xt_pool = ph2.enter_context(tc.tile_pool(name="xt", bufs=1))
            xt_all = xt_pool.tile([128, n_ct * t], BF16, tag="xt", name="xt")

            consts = ph2.enter_context(tc.tile_pool(name="consts", bufs=1))
            ident = consts.tile([128, 128], F32)
            nc.vector.memset(ident[:].bitcast(U32), 0)
            nc.gpsimd.affine_select(
                out=ident[:],
                in_=ident[:],
                compare_op=mybir.AluOpType.not_equal,
                fill=1.0,
                base=0,
                pattern=[[-1, 128]],
                channel_multiplier=1,
            )
            nc.vector.memset(ones_sb[:].bitcast(U32), 0x3F800000)
            # pre-fill V tiles with 1.0 so the padding column (softmax
            # denominator ones-row) survives; the V epilogue overwrites the
            # data columns. Emitted after ident so the first transpose isn't
            # stuck behind these memsets on the DVE queue.
            for vt in v_tiles:
                nc.vector.memset(vt[:].bitcast(U32), 0x3F803F80)
            bq_sb = consts.tile([128, n_dt], F32)
            bk_sb = consts.tile([128, n_dt], F32)
            nc.gpsimd.dma_start(bq_sb[:], bq_d)
            nc.gpsimd.dma_start(bk_sb[:], bk_d)
            bv_sb = consts.tile([128, DG], F32)
            nc.gpsimd.dma_start(
                bv_sb[:],
                bass.AP(tensor=bv_d.tensor, offset=0, ap=[[0, 128], [1, DG]]),
            )

            tab_pool = ph2.enter_context(tc.tile_pool(name="tab", bufs=1))
            cos_sb = tab_pool.tile([128, t], BF16)
            sin_sb = tab_pool.tile([128, t], BF16)

            # QK weight slabs (full width, contiguous rows -> few efficient DMAs)
            wqk_pool = ph2.enter_context(tc.tile_pool(name="wqk", bufs=1))
            wq_sb = [wqk_pool.tile([128, DG], BF16, tag=f"wq{i}", name=f"wq{i}") for i in range(n_ct)]
            wk_sb = [wqk_pool.tile([128, DG], BF16, tag=f"wk{i}", name=f"wk{i}") for i in range(n_ct)]

            # shared F32 psum ring: transposes, V psums, QK psums, phase-C bc
            psW = ph2.enter_context(tc.tile_pool(name="psW", bufs=2, space="PSUM"))
            xa_pool = ph2.enter_context(tc.tile_pool(name="xa", bufs=3))

            def bc_alloc_C():
                return psW.tile([128, NB], F32, tag="w", name="bcC")

            # ---- phase B: load x, transpose into xT, V matmuls per tile ----
            with ExitStack() as phB:
                wv_pool = phB.enter_context(tc.tile_pool(name="wv", bufs=1))
                wv_sb = [
                    wv_pool.tile([128, DG], BF16, tag=f"wv{i}", name=f"wv{i}")
                    for i in range(n_ct)
                ]
                for ci in range(n_ct):
                    nc.gpsimd.dma_start(wv_sb[ci][:], wv_d[ci * 128 : (ci + 1) * 128, :])
                # RoPE tables after wv (gpsimd DMA casts f32->bf16)
                nc.gpsimd.dma_start(cos_sb[:], cos_d)
                nc.gpsimd.dma_start(sin_sb[:], sin_d)

                for ti in range(n_tt):
                    xa = xa_pool.tile([128, C], F32, tag="xa", name="xa")
                    nc.sync.dma_start(xa[:], x_d[ti * 128 : (ti + 1) * 128, :])
                    # spread weight prefetches across phase B on the sync queue
                    if ti < 8:
                        nc.sync.dma_start(
                            wq_sb[ti][:], wq_d[ti * 128 : (ti + 1) * 128, :]
                        )
                        nc.sync.dma_start(
                            wk_sb[ti][:], wk_d[ti * 128 : (ti + 1) * 128, :]
                        )
                    elif ti < 12:
                        ci = ti - 8
                        nc.sync.dma_start(
                            wp_sb[ci][:], wp_d[ci * 128 : (ci + 1) * 128, :]
                        )
                    for half in range(2):
                        tp = psW.tile([128, 512], F32, tag="w", name="tp")
                        for k in range(4):
                            ci = half * 4 + k
                            nc.tensor.transpose(
                                tp[:, k * 128 : (k + 1) * 128],
                                xa[:, ci * 128 : (ci + 1) * 128],
                                ident[:],
                            )
                        base = xt_all[:]
                        dst = bass.AP(
                            tensor=base.tensor,
                            offset=base.offset + (half * 4) * t + ti * 128,
                            ap=[list(base.ap[0]), [t, 4], [1, 128]],
                        )
                        nc.scalar.copy(
                            dst, tp[:].rearrange("p (g c) -> p g c", g=4)
                        )
                    ps = psW.tile([128, DG], F32, tag="w", name="ps2v")
                    for ci in range(n_ct):
                        nc.tensor.matmul(
                            ps[:],
                            xt_all[:, ci * t + ti * 128 : ci * t + (ti + 1) * 128],
                            wv_sb[ci][:],
                            start=(ci == 0),
                            stop=(ci == n_ct - 1),
                        )
                    vt = v_tiles[ti]
                    dst = bass.AP(
                        tensor=vt[:].tensor,
                        offset=vt[:].offset,
                        ap=[list(vt[:].ap[0]), [HD + 1, HG], [1, HD]],
                    )
                    nc.vector.tensor_add(
                        dst,
                        ps[:].rearrange("p (h d) -> p h d", h=HG),
                        bv_sb[:].rearrange("p (h d) -> p h d", h=HG),
                    )

            # ---- phase C: Q^T, K^T + RoPE; hp0 attention pass interleaved ----
            rope_tmp = ph2.enter_context(tc.tile_pool(name="rtmp", bufs=2))
            shuf_mask = _pair_swap_mask()

            def qk_block(w_sb, b_sb, dst, dt_i, units_per_chain, max_hp):
                wts = [
                    w_sb[ci][:, dt_i * 128 : (dt_i + 1) * 128] for ci in range(n_ct)
                ]
                for nb_i in range(n_tb):
                    ps = psW.tile([128, NB], F32, tag="w", name="ps2")
                    for ci in range(n_ct):
                        nc.tensor.matmul(
                            ps[:],
                            wts[ci],
                            xt_all[:, ci * t + nb_i * NB : ci * t + (nb_i + 1) * NB],
                            start=(ci == 0),
                            stop=(ci == n_ct - 1),
                        )
                    nc.vector.tensor_scalar_add(
                        dst[dt_i][:, nb_i * NB : (nb_i + 1) * NB],
                        ps[:],
                        b_sb[:, dt_i : dt_i + 1],
                    )
                    emit_units(units_per_chain, bc_alloc_C, max_hp)

            def rope(q):
                tmp = rope_tmp.tile([128, t], BF16, tag="rtmp", name="rtmp")
                nc.vector.stream_shuffle(
                    tmp[:].bitcast(U32), q[:].bitcast(U32), shuf_mask
                )
                nc.vector.tensor_mul(tmp[:], tmp[:], sin_sb[:])
                nc.vector.tensor_mul(q[:], q[:], cos_sb[:])
                nc.vector.tensor_add(q[:], q[:], tmp[:])

            for dt_i in range(n_dt):
                upc = (0, 2, 3, 1)[dt_i]
                qk_block(wq_sb, bq_sb, qt_tiles, dt_i, upc, dt_i - 1)
                qk_block(wk_sb, bk_sb, kt_tiles, dt_i, upc, dt_i - 1)
                rope(qt_tiles[dt_i])
                rope(kt_tiles[dt_i])
            # drain the rest of the hp0 pass
            emit_units(max(0, n_hp0 - ucur["i"]), bc_alloc_C, 99)

        # ------- post C: remaining units + projection -------
        with ExitStack() as ph3:
            ps_pp = ph3.enter_context(tc.tile_pool(name="ps_pp", bufs=2, space="PSUM"))
            o_pool = ph3.enter_context(tc.tile_pool(name="o", bufs=3))

            def bc_alloc_3():
                return ps_pp.tile([128, NB], F32, tag="pp", name="bc3")

            def emit_proj_chain(ib, k, nb_i, oeng=None):
                ti = ib * (NB // 128) + k
                pp = ps_pp.tile([128, NB], F32, tag="pp", name="pp")
                for ci in range(n_cy):
                    nc.tensor.matmul(
                        pp[:],
                        yt_tiles[ci][:, ti * 128 : (ti + 1) * 128],
                        wp_sb[ci][:, nb_i * NB : (nb_i + 1) * NB],
                        start=(ci == 0),
                        stop=(ci == n_cy - 1),
                    )
                o_sb = o_pool.tile([128, NB], F32, tag="o", name="o")
                nc.vector.tensor_copy(o_sb[:], pp[:])
                (oeng or nc.sync).dma_start(
                    out_d[
                        ti * 128 : (ti + 1) * 128,
                        nb_i * NB : (nb_i + 1) * NB,
                    ],
                    o_sb[:],
                )

            pending = []

            def filler():
                if pending:
                    emit_proj_chain(*pending.pop(0))

            while ucur["i"] < len(units):
                u = ucur["i"]
                emit_units(1, bc_alloc_3, 99, filler)
                if u in proj_after:
                    ib = proj_after[u]
                    pending.extend(
                        (ib, k, nb_i)
                        for k in range(NB // 128)
                        for nb_i in range(C // NB)
                    )
            for i, ch in enumerate(pending):
                emit_proj_chain(*ch, oeng=(nc.sync if i % 2 == 0 else nc.scalar))

    nc.compile()
    _NC_CACHE[key] = nc
    return nc


def _rope_tables(t):
    """cos/sin in interleaved layout; sin sign-folded. Matches jax fp32."""
    inv_freq = (
        1.0 / (10000.0 ** (np.arange(0, HD, 2, dtype=np.float32) / np.float32(HD)))
    ).astype(np.float32)
    tt = np.arange(t, dtype=np.float32)
    freqs = (tt[:, None] * inv_freq[None, :]).astype(np.float32)  # [t, 32]
    cos_t = np.cos(freqs).astype(np.float32)  # [t, 32]
    sin_t = np.sin(freqs).astype(np.float32)
    cos64 = np.empty((64, t), dtype=np.float32)
    sinS64 = np.empty((64, t), dtype=np.float32)
    cos64[0::2] = cos_t.T
    cos64[1::2] = cos_t.T
    sinS64[0::2] = -sin_t.T
    sinS64[1::2] = sin_t.T
    cosT = np.concatenate([cos64, cos64], axis=0)  # [128, t]
    sinS = np.concatenate([sinS64, sinS64], axis=0)
    return np.ascontiguousarray(cosT), np.ascontiguousarray(sinS)


def _ilv_perm():
    """Interleave permutation within a head: new[2i]=old[i], new[2i+1]=old[32+i]."""
    p = np.empty(HD, dtype=np.int64)
    p[0::2] = np.arange(32)
    p[1::2] = np.arange(32, 64)
    return p


def kernel(x, w_attn, b_attn, w_proj, b_proj):
    x = np.asarray(x, dtype=np.float32)
    w_attn = np.asarray(w_attn, dtype=np.float32)
    b_attn = np.asarray(b_attn, dtype=np.float32)
    w_proj = np.asarray(w_proj, dtype=np.float32)
    b_proj = np.asarray(b_proj, dtype=np.float32)

    t = x.shape[1]
    nc = build_nc(t)

    ilv = _ilv_perm()
    cosT, sinS = _rope_tables(t)

    in_maps = []
    for c in range(8):
        b = c // 2
        g = c % 2
        heads = np.arange(HG * g, HG * (g + 1))
        qcols = np.concatenate([h * HD + ilv for h in heads])
        wq = np.ascontiguousarray(w_attn[:, qcols]).astype(ml_dtypes.bfloat16)
        wk = np.ascontiguousarray(w_attn[:, C + qcols]).astype(ml_dtypes.bfloat16)
        vcols = np.arange(2 * C + g * DG, 2 * C + (g + 1) * DG)
        wv = np.ascontiguousarray(w_attn[:, vcols]).astype(ml_dtypes.bfloat16)
        bq = np.ascontiguousarray(b_attn[qcols].reshape(-1, 128).T)
        bk = np.ascontiguousarray(b_attn[C + qcols].reshape(-1, 128).T)
        bv = np.ascontiguousarray(b_attn[vcols])
        wp = np.ascontiguousarray(w_proj[g * DG : (g + 1) * DG, :]).astype(ml_dtypes.bfloat16)
        in_maps.append(
            {
                "x": np.ascontiguousarray(x[b]),
                "wq": wq,
                "wk": wk,
                "wv": wv,
                "bq": bq,
                "bk": bk,
                "bv": bv,
                "wp": wp,
                "cosT": cosT,
                "sinS": sinS,
            }
        )

    res = run_bass_kernel_spmd(nc, in_maps, core_ids=list(range(8)))
    global LAST_RESULTS
    LAST_RESULTS = res

    out = np.empty((B, t, C), dtype=np.float32)
    for b in range(B):
        acc = (
            res.results[2 * b]["out"].astype(np.float64)
            + res.results[2 * b + 1]["out"].astype(np.float64)
            + b_proj.astype(np.float64)[None, :]
        )
        out[b] = acc.astype(np.float32)
    return out
